# revision 11
# baseline (speedup 1.0000x reference)
"""GATv2Conv TRN2 kernel v3 (8-core SPMD, src-sharded edges, fp16 tab).

Deltas vs v2 baseline:
  - tab rows fp16 [h(64)|s_src(4)|pad] = 256B (was f32 512B): halves
    h-gather bytes and SBUF.
  - Dense shared packing: per-window columns = max over cores of
    ceil(count/128) (was uniform B=3): ~25% fewer slots.
  - One dma_gather op per stream per chunk (nidx up to ~5.6k, was 896):
    Pool-engine DGE time ~6x lower.
  - fp16 one-hot + fp16 pay matmuls (f32 PSUM accum), fp16 acc + RS.
Structure otherwise follows v2: per-edge s_dst via sd256 chunk-block
gather; one-hot PE scatter into PSUM per window; fp16 acc blocks;
segmented ReduceScatter; final div.
"""
import math
import os
import time
from contextlib import ExitStack
from dataclasses import dataclass, field

import numpy as np

import concourse.bass as bass
import concourse.bacc as bacc
import concourse.mybir as mybir
import concourse.tile as tile
from concourse import bass_utils

F32 = mybir.dt.float32
F16 = mybir.dt.float16
I16 = mybir.dt.int16

N_NODES = 100000
N_EDGES = 1600000
HEADS = 4
HEAD_DIM = 16
EPS = 1e-8
NEG = 0.2
IN_CH = 128
ELEM = 128          # fp16 elems per tab row (256B)

LAST_EXEC_NS = None
LAST_NC = None
LAST_IN_MAPS = None
LAST_PLAN = None


@dataclass
class Plan:
    cores: int = 8
    nloc: int = 12544          # nodes per core (128-aligned src shard)
    wins: int = 784            # global dst windows
    MW: int = 14               # max windows per chunk
    KC: int = 44               # max columns per chunk
    PG: int = 7                # windows per PSUM group
    p1g: int = 7               # phase-1 permutation group (98 = 14*7)
    chunks: list = field(default_factory=list)  # [(w0, [ncols...])]
    colstart: np.ndarray = None    # [wins] global col index of window
    chunk_cols: list = field(default_factory=list)
    tcols: int = 0
    nchunks_p: int = 0         # padded to multiple of 8

    @property
    def lwins(self):
        return self.nloc // 128  # 98

    @property
    def acc_rows(self):
        return self.nchunks_p * self.MW * 128

    @property
    def red_rows(self):
        return self.acc_rows // self.cores


def _make_ap(base_ap, rel_offset, dims):
    return bass.AP(base_ap.tensor, base_ap.offset + rel_offset,
                   [list(d) for d in dims])


def _bcast_dim(ap_obj, insert_at, count):
    newap = [list(x) for x in ap_obj.ap]
    newap.insert(insert_at, [0, count])
    return bass.AP(ap_obj.tensor, ap_obj.offset, newap)


def _wrap16(arr2d):
    """[128, cols] col-major flat -> [16, n/16] wrapped, replicated x8."""
    flat = arr2d.T.ravel()                      # i = col*128 + row
    w = flat.reshape(-1, 16).T                  # [16, n/16]
    return np.tile(w, (8, 1))                   # [128, n/16]


def _host_prep(P, x, edge_index, edge_weight, W, a):
    src = np.asarray(edge_index[0], dtype=np.int64)
    dst = np.asarray(edge_index[1], dtype=np.int64)
    w = np.asarray(edge_weight, dtype=np.float32)

    core = np.minimum(src // P.nloc, P.cores - 1)
    win = dst >> 7
    dstp = (dst & 127).astype(np.int64)

    # local src id -> permuted tab row (phase-1 write locality)
    nl = src - core * P.nloc
    wl, pl = nl >> 7, nl & 127
    loc_src = (wl // P.p1g) * (P.p1g * 128) + pl * P.p1g + (wl % P.p1g)

    # per (core, win) counts -> shared per-window column counts
    group = core * P.wins + win
    counts = np.bincount(group, minlength=P.cores * P.wins)
    counts = counts.reshape(P.cores, P.wins)
    cols_w = np.maximum((counts.max(axis=0) + 127) // 128, 1)  # [wins]

    # greedy chunking: consecutive windows, <=MW windows, <=KC columns
    chunks = []
    cur_w0, cur_cols = 0, []
    for wdx in range(P.wins):
        c = int(cols_w[wdx])
        if cur_cols and (len(cur_cols) >= P.MW or sum(cur_cols) + c > P.KC):
            chunks.append((cur_w0, cur_cols))
            cur_w0, cur_cols = wdx, []
        cur_cols.append(c)
    chunks.append((cur_w0, cur_cols))
    P.chunks = chunks
    P.chunk_cols = [sum(cl) for _, cl in chunks]
    P.tcols = int(sum(P.chunk_cols))
    P.nchunks_p = ((len(chunks) + P.cores - 1) // P.cores) * P.cores

    # global column start per window + window offset within its chunk
    colstart = np.zeros(P.wins, dtype=np.int64)
    joff = np.zeros(P.wins, dtype=np.int64)
    cb = 0
    for w0, cl in chunks:
        for j, c in enumerate(cl):
            colstart[w0 + j] = cb
            joff[w0 + j] = j
            cb += c
    P.colstart = colstart

    # per-edge placement
    if os.environ.get("K3_SRCSORT", "0") == "1":
        order = np.lexsort((loc_src, group))
    else:
        order = np.argsort(group, kind="stable")
    g_sorted = group[order]
    starts = np.zeros(P.cores * P.wins, dtype=np.int64)
    np.cumsum(counts.ravel()[:-1], out=starts[1:])
    iw = np.arange(len(src), dtype=np.int64) - starts[g_sorted]

    core_s = g_sorted // P.wins
    win_s = g_sorted % P.wins
    rows = iw & 127
    cols = colstart[win_s] + (iw >> 7)

    sh = (P.cores, 128, P.tcols)
    idxg = np.zeros(sh, dtype=np.int16)
    dstc = np.full(sh, -1.0, dtype=np.float16)
    dstc8 = np.full(sh, -1, dtype=np.int8)
    wc = np.zeros(sh, dtype=np.float16)
    idxg[core_s, rows, cols] = loc_src[order].astype(np.int16)
    dstc[core_s, rows, cols] = dstp[order].astype(np.float16)
    dstc8[core_s, rows, cols] = dstp[order].astype(np.int8)
    wc[core_s, rows, cols] = w[order].astype(np.float16)
    # dstcT: [128(bcast), tcols*128] int8; value at (d, c*128+p) = dstc[p, c]
    dstcT = np.ascontiguousarray(np.broadcast_to(
        dstc8.transpose(0, 2, 1).reshape(P.cores, 1, P.tcols * 128),
        (P.cores, 128, P.tcols * 128)))

    # pack gather indices (h-stream only), chunk-contiguous
    i16_per_col = 8
    idxp = np.empty((P.cores, 128, P.tcols * i16_per_col), dtype=np.int16)
    for c in range(P.cores):
        off = 0
        cb = 0
        for ci, (_w0, cl) in enumerate(chunks):
            nc_ = sum(cl)
            n16 = nc_ * i16_per_col
            idxp[c][:, off:off + n16] = _wrap16(idxg[c][:, cb:cb + nc_])
            off += n16
            cb += nc_

    xf = np.asarray(x, dtype=np.float32)
    xts = []
    for c in range(P.cores):
        lo = c * P.nloc
        hi = min((c + 1) * P.nloc, N_NODES)
        xt = np.zeros((IN_CH, P.nloc), dtype=np.float32)
        xt[:, :hi - lo] = xf[lo:hi].T
        xts.append(xt)

    Wt = np.ascontiguousarray(np.asarray(W, dtype=np.float32).T)  # [128,64]
    a_np = np.asarray(a, dtype=np.float32)
    a_src = a_np[0, :, :HEAD_DIM]
    a_dst = a_np[0, :, HEAD_DIM:]
    A_src = (Wt.reshape(IN_CH, HEADS, HEAD_DIM) * a_src[None]).sum(-1)
    A_dst = (Wt.reshape(IN_CH, HEADS, HEAD_DIM) * a_dst[None]).sum(-1)
    rhs_ext = np.ascontiguousarray(
        np.concatenate([Wt, A_src, A_dst], axis=1), dtype=np.float32)
    iota = np.ascontiguousarray(np.broadcast_to(
        np.arange(128, dtype=np.float16), (128, 128)))
    iotap = np.ascontiguousarray(
        np.arange(128, dtype=np.float16).reshape(128, 1))

    in_maps = []
    for c in range(P.cores):
        in_maps.append(dict(
            xT=xts[c], rhs_ext=rhs_ext, iota=iota, iotap=iotap,
            idxp=idxp[c], dstc=dstc[c], dstcT=dstcT[c], wc=wc[c]))
    return in_maps


def _build_program(P, num_devices=None, qmap=None):
    REPS = int(os.environ.get("K3_REPS", "1"))
    ABL = set(os.environ.get("K3_ABL", "").split(","))
    REPCC = int(os.environ.get("K3_REPCC", "1"))
    ACCD = F16 if os.environ.get("K3_F16ACC", "1") == "1" else F32
    ND = num_devices or P.cores
    MW, PG, KC = P.MW, P.PG, P.KC
    nch = len(P.chunks)
    scratch = int(os.environ.get("K3_SCRATCH", "49152"))

    nc = bacc.Bacc("TRN2", target_bir_lowering=False, debug=False,
                   enable_asserts=False, num_devices=ND,
                   dynamic_dma_scratch_size=scratch, num_swdge_queues=int(os.environ.get("K3_NSWQ", "4")))
    xT_d = nc.dram_tensor("xT", [IN_CH, P.nloc], F32, kind="ExternalInput")
    re_d = nc.dram_tensor("rhs_ext", [IN_CH, 72], F32, kind="ExternalInput")
    io_d = nc.dram_tensor("iota", [128, 128], F16, kind="ExternalInput")
    idxp_d = nc.dram_tensor("idxp", [128, P.tcols * 16], I16,
                            kind="ExternalInput")
    dstc_d = nc.dram_tensor("dstc", [128, P.tcols], F16,
                            kind="ExternalInput")
    wc_d = nc.dram_tensor("wc", [128, P.tcols], F16, kind="ExternalInput")

    tab_d = nc.dram_tensor("tab", [P.nloc, ELEM], F16, kind="Internal")
    sdT_d = nc.dram_tensor("sdT", [128, P.lwins * 4], F32, kind="Internal")
    sdF_d = nc.dram_tensor("sdF", [128 * ND, P.lwins * 4], F32,
                           kind="Internal", addr_space="Shared")
    sd256_d = nc.dram_tensor("sd256", [P.nchunks_p * MW * 128, 64], F32,
                             kind="Internal")
    acc_d = nc.dram_tensor("acc", [P.acc_rows, 68], ACCD, kind="Internal")
    red_d = nc.dram_tensor("red", [P.red_rows, 68], ACCD, kind="Internal")
    out_d = nc.dram_tensor("out", [P.red_rows, 64], F32,
                           kind="ExternalOutput")

    groups = [list(range(ND))]

    with tile.TileContext(nc) as tc, ExitStack() as ctx:
        const = ctx.enter_context(tc.tile_pool(name="const", bufs=1))
        iota_t = const.tile([128, 128], F16)
        nc.sync.dma_start(out=iota_t[:], in_=io_d[:])
        re_t = const.tile([128, 72], F32)
        nc.sync.dma_start(out=re_t[:], in_=re_d[:])
        dstc_t = const.tile([128, P.tcols], F16)
        nc.sync.dma_start(out=dstc_t[:], in_=dstc_d[:])
        wc_t = const.tile([128, P.tcols], F16)
        nc.sync.dma_start(out=wc_t[:], in_=wc_d[:])
        sdall = const.tile([128, ND, P.lwins * 4], F32)

        # ---- phase 1: tab rows (permuted) = [h | s_src | junk] fp16 ----
        with tc.tile_pool(name="xload", bufs=1) as xp, \
             tc.tile_pool(name="hstage", bufs=3) as hp, \
             tc.tile_pool(name="psh", bufs=2, space="PSUM") as php:
            xt = xp.tile([128, P.nloc], F32, tag="xt")
            nc.sync.dma_start(out=xt[:], in_=xT_d[:])
            sdstage = xp.tile([128, P.lwins, 4], F32, tag="sds")
            for g7 in range(P.lwins // P.p1g):
                ph = php.tile([128, P.p1g, 72], F32, tag="ph")
                for j in range(P.p1g):
                    jw = g7 * P.p1g + j
                    nc.tensor.matmul(
                        out=ph[:, j, :], lhsT=xt[:, jw * 128:(jw + 1) * 128],
                        rhs=re_t[:], start=True, stop=True)
                hs = hp.tile([128, P.p1g, ELEM], F16, tag="hs")
                nc.vector.memset(hs[:, :, 68:ELEM], 0.0)
                nc.vector.tensor_copy(out=hs[:, :, 0:68],
                                      in_=ph[:, :, 0:68])
                nc.vector.tensor_copy(
                    out=sdstage[:, g7 * P.p1g:(g7 + 1) * P.p1g, :],
                    in_=ph[:, :, 68:72])
                dst_ap = _make_ap(
                    tab_d[:], g7 * P.p1g * 128 * ELEM,
                    [[P.p1g * ELEM, 128], [1, P.p1g * ELEM]])
                nc.sync.dma_start(
                    out=dst_ap, in_=hs[:].rearrange("p j e -> p (j e)"))
            nc.sync.dma_start(
                out=sdT_d[:],
                in_=sdstage[:].rearrange("p j e -> p (j e)"))

        nc.gpsimd.collective_compute(
            "AllGather", mybir.AluOpType.bypass, groups,
            ins=[sdT_d[:]], outs=[sdF_d[:]])
        src_ap = _make_ap(sdF_d[:], 0,
                          [[P.lwins * 4, 128], [128 * P.lwins * 4, ND],
                           [1, P.lwins * 4]])
        nc.sync.dma_start(out=sdall[:], in_=src_ap)

        # sd256 chunk blocks: row p*MW + j = s_dst(node w0+j @ part p)
        with tc.tile_pool(name="sdb", bufs=3) as sdb:
            for ci, (w0, cl) in enumerate(P.chunks):
                nw = len(cl)
                st = sdb.tile([128, MW, 64], F32, tag="st")
                nc.vector.memset(st[:], 0.0)
                j = 0
                while j < nw:
                    wg = w0 + j
                    k8, jw = divmod(wg, P.lwins)
                    span = min(nw - j, P.lwins - jw)
                    sap = _make_ap(
                        sdall[:], k8 * P.lwins * 4 + jw * 4,
                        [list(sdall[:].ap[0]), [4, span], [1, 4]])
                    nc.vector.tensor_copy(out=st[:, j:j + span, 0:4],
                                          in_=sap)
                    j += span
                dst_ap = _make_ap(
                    sd256_d[:], ci * MW * 128 * 64,
                    [[MW * 64, 128], [1, MW * 64]])
                nc.sync.dma_start(
                    out=dst_ap, in_=st[:].rearrange("p w e -> p (w e)"))

        # ---------------- phase 2: edges ----------------
        p2ctx = ExitStack()
        sbi = p2ctx.enter_context(tc.tile_pool(name="edgei", bufs=3))
        sbg = p2ctx.enter_context(tc.tile_pool(name="edgeg", bufs=3))
        sbs = p2ctx.enter_context(tc.tile_pool(name="edges", bufs=2))
        wb = p2ctx.enter_context(tc.tile_pool(name="winb", bufs=2))
        psa = p2ctx.enter_context(tc.tile_pool(name="psa", bufs=2,
                                               space="PSUM"))
        pse = p2ctx.enter_context(tc.tile_pool(name="pse", bufs=2,
                                               space="PSUM"))

        QROT = int(os.environ.get("K3_QROT", "0"))
        NQ = int(os.environ.get("K3_NSWQ", "4"))
        swq = [0]
        for _rp in range(REPS if "nophase2" not in ABL else 0):
            colptr = 0
            idxoff = 0
            for ci, (w0, cl) in enumerate(P.chunks):
                nw = len(cl)
                cols_c = sum(cl)
                nidx = cols_c * 128
                n16 = cols_c * 8

                idx_t = sbi.tile([128, KC * 8], I16, tag="idx")
                nc.sync.dma_start(
                    out=idx_t[:, 0:n16],
                    in_=idxp_d[:, idxoff:idxoff + n16])
                dct = sbi.tile([128, KC, 128], mybir.dt.int8, tag="dct")
                nc.sync.dma_start(
                    out=dct[:, 0:cols_c, :].rearrange("p c d -> p (c d)"),
                    in_=dstcT_d[:, colptr * 128:(colptr + cols_c) * 128])
                GOPC = int(os.environ.get("K3_GOPC", "22")) or cols_c
                g = sbg.tile([128, KC, ELEM], F16, tag="g")
                if "nogather" in ABL:
                    nc.vector.memset(g[:, 0:1, 0:4], 0.0)
                for o0 in range(0, cols_c if "nogather" not in ABL else 0,
                                GOPC):
                    on = min(GOPC, cols_c - o0)
                    nc.gpsimd.dma_gather(
                        g[:, o0:o0 + on, :], tab_d[:],
                        idx_t[:, o0 * 8:(o0 + on) * 8],
                        on * 128, on * 128, ELEM,
                        queue_num=(qmap[swq[0] % len(qmap)] if qmap
                                   else (swq[0] * QROT) % NQ),
                        single_packet=False)
                    swq[0] += 1
                # ohT[d, c, p] = (dstc[p, c] == d); expansion matmuls give
                # per-edge s_dst in PSUM: sde_ps[p, c, :] = sdall[dstc[p,c]]
                ohT = sbs.tile([128, KC, 128], F16, tag="ohT")
                sde_ps = pse.tile([128, KC, 4], F32, tag="sdeps")
                if "noexp" in ABL:
                    nc.vector.memset(ohT[:, 0:1, 0:4], 0.0)
                    nc.vector.memset(sde_ps[:, 0:1, :], 0.0)
                else:
                    nc.vector.tensor_tensor(
                        out=ohT[:, 0:cols_c, :],
                        in0=iop_t[:].to_broadcast([128, cols_c, 128]),
                        in1=dct[:, 0:cols_c, :],
                        op=mybir.AluOpType.is_equal)
                    cb2 = 0
                    for j2, ncw2 in enumerate(cl):
                        wg = w0 + j2
                        k8, jw = divmod(wg, P.lwins)
                        for j3 in range(ncw2):
                            nc.tensor.matmul(
                                out=sde_ps[:, cb2 + j3, :],
                                lhsT=ohT[:, cb2 + j3, :],
                                rhs=sdallh[:, k8, jw * 4:(jw + 1) * 4],
                                start=True, stop=True)
                        cb2 += ncw2

                oh = wb.tile([128, KC, 128], F16, tag="oh")
                if "nooh" in ABL:
                    nc.vector.memset(oh[:, 0:1, 0:4], 0.0)
                else:
                    nc.vector.tensor_tensor(
                        out=oh[:, 0:cols_c, :],
                        in0=_bcast_dim(iota_t[:], 1, cols_c),
                        in1=dstc_t[:, colptr:colptr + cols_c].to_broadcast(
                            [128, cols_c, 128]),
                        op=mybir.AluOpType.is_equal)

                logit = wb.tile([128, KC, 4], F16, tag="logit")
                nc.vector.tensor_add(
                    out=logit[:, 0:cols_c, :],
                    in0=g[:, 0:cols_c, 64:68],
                    in1=sde_ps[:, 0:cols_c, :])
                nc.vector.scalar_tensor_tensor(
                    out=logit[:, 0:cols_c, :], in0=logit[:, 0:cols_c, :],
                    scalar=NEG, in1=logit[:, 0:cols_c, :],
                    op0=mybir.AluOpType.mult, op1=mybir.AluOpType.max)
                nc.vector.tensor_mul(
                    out=logit[:, 0:cols_c, :], in0=logit[:, 0:cols_c, :],
                    in1=wc_t[:, colptr:colptr + cols_c].to_broadcast(
                        [128, cols_c, 4]))
                p = wb.tile([128, KC, 4], F16, tag="p")
                if "noact" in ABL:
                    nc.vector.tensor_copy(out=p[:, 0:cols_c, :],
                                          in_=logit[:, 0:cols_c, :])
                else:
                    nc.scalar.activation(p[:, 0:cols_c, :],
                                         logit[:, 0:cols_c, :],
                                         mybir.ActivationFunctionType.Exp)

                pay = wb.tile([128, KC, 68], F16, tag="pay")
                pv = p[:, 0:cols_c, :].to_broadcast([128, cols_c, 4, 16])
                gv = g[:, 0:cols_c, 0:64].rearrange(
                    "p k (h d) -> p k h d", d=16)
                ov = pay[:, 0:cols_c, 0:64].rearrange(
                    "p k (h d) -> p k h d", d=16)
                nc.vector.tensor_mul(out=ov, in0=gv, in1=pv)
                nc.vector.tensor_copy(out=pay[:, 0:cols_c, 64:68],
                                      in_=p[:, 0:cols_c, :])

                stage = wb.tile([128, MW, 68], ACCD, tag="stage")
                if nw < MW:
                    nc.vector.memset(stage[:, nw:MW, :], 0.0)
                for g0 in range(0, nw, PG):
                    gn = min(PG, nw - g0)
                    acc_ps = psa.tile([128, PG, 68], F32, tag="acc")
                    cb = sum(cl[:g0])
                    if "nomm" in ABL:
                        nc.vector.memset(acc_ps[:, 0:1, 0:4], 0.0)
                    for wl in range(gn if "nomm" not in ABL else 0):
                        ncw = cl[g0 + wl]
                        for j in range(ncw):
                            nc.tensor.matmul(
                                out=acc_ps[:, wl, :],
                                lhsT=oh[:, cb + j, :],
                                rhs=pay[:, cb + j, :],
                                start=(j == 0), stop=(j == ncw - 1))
                        cb += ncw
                    nc.vector.tensor_copy(
                        out=stage[:, g0:g0 + gn, :],
                        in_=acc_ps[:, 0:gn, :])
                dst_ap = _make_ap(
                    acc_d[:], ci * MW * 128 * 68,
                    [[MW * 68, 128], [1, MW * 68]])
                nc.scalar.dma_start(
                    out=dst_ap, in_=stage[:].rearrange("p w e -> p (w e)"))
                colptr += cols_c
                idxoff += n16

        p2ctx.close()

        # ---------------- ReduceScatter ----------------
        SEGB = ND * MW * 128
        OUTB = MW * 128
        nseg = P.acc_rows // SEGB
        for _rc in range(REPCC):
            if ND == 1:
                nc.sync.dma_start(out=red_d[:], in_=acc_d[0:P.red_rows, :])
            else:
                for s in range(nseg):
                    nc.gpsimd.collective_compute(
                        "ReduceScatter", mybir.AluOpType.add, groups,
                        ins=[acc_d[s * SEGB:(s + 1) * SEGB, :]],
                        outs=[red_d[s * OUTB:(s + 1) * OUTB, :]])

        # ---------------- final: out = num / (den + eps) ----------------
        with tc.tile_pool(name="fin", bufs=2) as fp:
            for fb in range(P.red_rows // (MW * 128)):
                src2 = _make_ap(red_d[:], fb * MW * 128 * 68,
                                [[MW * 68, 128], [1, MW * 68]])
                rt = fp.tile([128, MW, 68], ACCD, tag="rt")
                nc.sync.dma_start(
                    out=rt[:].rearrange("p w e -> p (w e)"), in_=src2)
                rec = fp.tile([128, MW, 4], F32, tag="rec")
                nc.vector.tensor_scalar_add(out=rec[:], in0=rt[:, :, 64:68],
                                            scalar1=EPS)
                nc.vector.reciprocal(out=rec[:], in_=rec[:])
                ot = fp.tile([128, MW, 64], F32, tag="ot")
                nc.vector.tensor_mul(
                    out=ot[:].rearrange("p k (h d) -> p k h d", d=16),
                    in0=rt[:, :, 0:64].rearrange("p k (h d) -> p k h d",
                                                 d=16),
                    in1=rec[:].to_broadcast([128, MW, 4, 16]))
                dst2 = _make_ap(out_d[:], fb * MW * 128 * 64,
                                [[MW * 64, 128], [1, MW * 64]])
                nc.sync.dma_start(
                    out=dst2, in_=ot[:].rearrange("p w e -> p (w e)"))

    nc.compile()
    return nc


def _gather_lane_qmap(nc):
    """Emission-ordered SWDGE lane assignment -> queue map (lane %% 4)."""
    from concourse.tile_scheduler import PROC_NAME_TO_IDX
    idx_to_proc = {v: k for k, v in PROC_NAME_TO_IDX.items()}
    lanes = []
    for blk in nc.m.functions[0].blocks:
        for ins in blk.instructions:
            if type(ins).__name__ == "InstDMAGatherAnt":
                proc = idx_to_proc.get(ins.bass_scheduled_proc, "")
                assert proc.startswith("DMASW"), proc
                lanes.append((int(ins.name.split("-")[1]),
                              int(proc[5:]) % 4))
    lanes.sort()
    return [q for _, q in lanes]


def build_program_tuned(P, num_devices=None):
    """Two-pass build: discover Tile's DMASW lane order, re-emit with
    queue_num matched to lane %% 4 (consistent sem/queue pairing at
    4-queue parallelism)."""
    nc0 = _build_program(P, num_devices)
    qmap = _gather_lane_qmap(nc0)
    return _build_program(P, num_devices, qmap=qmap)


def _build_base(P, num_devices=None):
    """I/O-identical near-empty program for dispatch-overhead calibration."""
    ND = num_devices or P.cores
    nc = bacc.Bacc("TRN2", target_bir_lowering=False, debug=False,
                   enable_asserts=False, num_devices=ND)
    xT_d = nc.dram_tensor("xT", [IN_CH, P.nloc], F32, kind="ExternalInput")
    nc.dram_tensor("rhs_ext", [IN_CH, 72], F32, kind="ExternalInput")
    nc.dram_tensor("iota", [128, 128], F16, kind="ExternalInput")
    nc.dram_tensor("idxp", [128, P.tcols * 8], I16, kind="ExternalInput")
    nc.dram_tensor("dstc", [128, P.tcols], F16, kind="ExternalInput")
    nc.dram_tensor("dstcT", [128, P.tcols * 128], mybir.dt.int8,
                   kind="ExternalInput")
    nc.dram_tensor("iotap", [128, 1], F16, kind="ExternalInput")
    nc.dram_tensor("wc", [128, P.tcols], F16, kind="ExternalInput")
    out_d = nc.dram_tensor("out", [P.red_rows, 64], F32,
                           kind="ExternalOutput")
    with tile.TileContext(nc) as tc, ExitStack() as ctx:
        sb = ctx.enter_context(tc.tile_pool(name="sb", bufs=1))
        f = sb.tile([128, 64], F32)
        nc.sync.dma_start(out=f[:], in_=xT_d[:, 0:64])
        nb = P.red_rows // 128
        dst = _make_ap(out_d[:], 0, [[64, 128], [0, nb], [1, 64]])
        nc.sync.dma_start(out=dst, in_=_bcast_dim(f[:], 1, nb))
    nc.compile()
    return nc


def _assemble(P, parts):
    """Per-core [red_rows, 64] outputs -> full [N_NODES, 64]."""
    MW = P.MW
    full = np.zeros((P.wins * 128, 64), dtype=parts[0].dtype)
    nb = P.nchunks_p // P.cores
    for k in range(P.cores):
        arr = parts[k].reshape(nb, 128, MW, 64)
        for s in range(nb):
            ci = P.cores * s + k
            if ci >= len(P.chunks):
                continue
            w0, cl = P.chunks[ci]
            nw = len(cl)
            blk = arr[s, :, 0:nw, :].transpose(1, 0, 2)   # [nw, 128, 64]
            full[w0 * 128:(w0 + nw) * 128] = blk.reshape(nw * 128, 64)
    return full[:N_NODES]


def kernel(x, edge_index, edge_weight, W, a):
    global LAST_EXEC_NS, LAST_NC, LAST_IN_MAPS, LAST_PLAN
    P = Plan()
    t0 = time.time()
    in_maps = _host_prep(P, x, edge_index, edge_weight, W, a)
    t1 = time.time()
    nc = build_program_tuned(P)
    LAST_NC = nc
    LAST_IN_MAPS = in_maps
    LAST_PLAN = P
    t2 = time.time()
    res = bass_utils.run_bass_kernel_spmd(
        nc, in_maps, core_ids=list(range(P.cores)))
    t3 = time.time()
    print(f"[kernel3] host_prep {t1-t0:.1f}s  build+compile {t2-t1:.1f}s  "
          f"exec(all-in) {t3-t2:.1f}s  tcols={P.tcols} nch={len(P.chunks)}")
    LAST_EXEC_NS = res.exec_time_ns
    parts = [res.results[c]["out"] for c in range(P.cores)]
    full = _assemble(P, parts)
    return np.ascontiguousarray(full)


# revision 12
# speedup vs baseline: 2.0282x; 2.0282x over previous
"""GATv2Conv TRN2 kernel v3 (8-core SPMD, src-sharded edges, fp16 tab).

Deltas vs v2 baseline:
  - tab rows fp16 [h(64)|s_src(4)|pad] = 256B (was f32 512B): halves
    h-gather bytes and SBUF.
  - Dense shared packing: per-window columns = max over cores of
    ceil(count/128) (was uniform B=3): ~25% fewer slots.
  - One dma_gather op per stream per chunk (nidx up to ~5.6k, was 896):
    Pool-engine DGE time ~6x lower.
  - fp16 one-hot + fp16 pay matmuls (f32 PSUM accum), fp16 acc + RS.
Structure otherwise follows v2: per-edge s_dst via sd256 chunk-block
gather; one-hot PE scatter into PSUM per window; fp16 acc blocks;
segmented ReduceScatter; final div.
"""
import math
import os
import time
from contextlib import ExitStack
from dataclasses import dataclass, field

import numpy as np

import concourse.bass as bass
import concourse.bacc as bacc
import concourse.mybir as mybir
import concourse.tile as tile
from concourse import bass_utils

F32 = mybir.dt.float32
F16 = mybir.dt.float16
I16 = mybir.dt.int16

N_NODES = 100000
N_EDGES = 1600000
HEADS = 4
HEAD_DIM = 16
EPS = 1e-8
NEG = 0.2
IN_CH = 128
ELEM = 128          # fp16 elems per tab row (256B)

LAST_EXEC_NS = None
LAST_NC = None
LAST_IN_MAPS = None
LAST_PLAN = None


@dataclass
class Plan:
    cores: int = 8
    nloc: int = 12544          # nodes per core (128-aligned src shard)
    wins: int = 784            # global dst windows
    MW: int = 14               # max windows per chunk
    KC: int = 44               # max columns per chunk
    PG: int = 7                # windows per PSUM group
    p1g: int = 7               # phase-1 permutation group (98 = 14*7)
    chunks: list = field(default_factory=list)  # [(w0, [ncols...])]
    colstart: np.ndarray = None    # [wins] global col index of window
    chunk_cols: list = field(default_factory=list)
    tcols: int = 0
    nchunks_p: int = 0         # padded to multiple of 8

    @property
    def lwins(self):
        return self.nloc // 128  # 98

    @property
    def acc_rows(self):
        return self.nchunks_p * self.MW * 128

    @property
    def red_rows(self):
        return self.acc_rows // self.cores


def _make_ap(base_ap, rel_offset, dims):
    return bass.AP(base_ap.tensor, base_ap.offset + rel_offset,
                   [list(d) for d in dims])


def _bcast_dim(ap_obj, insert_at, count):
    newap = [list(x) for x in ap_obj.ap]
    newap.insert(insert_at, [0, count])
    return bass.AP(ap_obj.tensor, ap_obj.offset, newap)


def _wrap16(arr2d):
    """[128, cols] col-major flat -> [16, n/16] wrapped, replicated x8."""
    flat = arr2d.T.ravel()                      # i = col*128 + row
    w = flat.reshape(-1, 16).T                  # [16, n/16]
    return np.tile(w, (8, 1))                   # [128, n/16]


def _host_prep(P, x, edge_index, edge_weight, W, a):
    src = np.asarray(edge_index[0], dtype=np.int64)
    dst = np.asarray(edge_index[1], dtype=np.int64)
    w = np.asarray(edge_weight, dtype=np.float32)

    core = np.minimum(src // P.nloc, P.cores - 1)
    win = dst >> 7
    dstp = (dst & 127).astype(np.int64)

    # local src id -> permuted tab row (phase-1 write locality)
    nl = src - core * P.nloc
    wl, pl = nl >> 7, nl & 127
    loc_src = (wl // P.p1g) * (P.p1g * 128) + pl * P.p1g + (wl % P.p1g)

    # per (core, win) counts -> shared per-window column counts
    group = core * P.wins + win
    counts = np.bincount(group, minlength=P.cores * P.wins)
    counts = counts.reshape(P.cores, P.wins)
    cols_w = np.maximum((counts.max(axis=0) + 127) // 128, 1)  # [wins]

    # greedy chunking: consecutive windows, <=MW windows, <=KC columns
    chunks = []
    cur_w0, cur_cols = 0, []
    for wdx in range(P.wins):
        c = int(cols_w[wdx])
        if cur_cols and (len(cur_cols) >= P.MW or sum(cur_cols) + c > P.KC):
            chunks.append((cur_w0, cur_cols))
            cur_w0, cur_cols = wdx, []
        cur_cols.append(c)
    chunks.append((cur_w0, cur_cols))
    P.chunks = chunks
    P.chunk_cols = [sum(cl) for _, cl in chunks]
    P.tcols = int(sum(P.chunk_cols))
    P.nchunks_p = ((len(chunks) + P.cores - 1) // P.cores) * P.cores

    # global column start per window + window offset within its chunk
    colstart = np.zeros(P.wins, dtype=np.int64)
    joff = np.zeros(P.wins, dtype=np.int64)
    cb = 0
    for w0, cl in chunks:
        for j, c in enumerate(cl):
            colstart[w0 + j] = cb
            joff[w0 + j] = j
            cb += c
    P.colstart = colstart

    # per-edge placement
    if os.environ.get("K3_SRCSORT", "0") == "1":
        order = np.lexsort((loc_src, group))
    else:
        order = np.argsort(group, kind="stable")
    g_sorted = group[order]
    starts = np.zeros(P.cores * P.wins, dtype=np.int64)
    np.cumsum(counts.ravel()[:-1], out=starts[1:])
    iw = np.arange(len(src), dtype=np.int64) - starts[g_sorted]

    core_s = g_sorted // P.wins
    win_s = g_sorted % P.wins
    rows = iw & 127
    cols = colstart[win_s] + (iw >> 7)

    sh = (P.cores, 128, P.tcols)
    idxg = np.zeros(sh, dtype=np.int16)
    dstc = np.full(sh, -1.0, dtype=np.float16)
    dstc8 = np.full(sh, -1, dtype=np.int8)
    wc = np.zeros(sh, dtype=np.float16)
    idxg[core_s, rows, cols] = loc_src[order].astype(np.int16)
    dstc[core_s, rows, cols] = dstp[order].astype(np.float16)
    dstc8[core_s, rows, cols] = dstp[order].astype(np.int8)
    wc[core_s, rows, cols] = w[order].astype(np.float16)
    # dstcT: [128(bcast), tcols*128] int8; value at (d, c*128+p) = dstc[p, c]
    dstcT = np.ascontiguousarray(np.broadcast_to(
        dstc8.transpose(0, 2, 1).reshape(P.cores, 1, P.tcols * 128),
        (P.cores, 128, P.tcols * 128)))

    # pack gather indices (h-stream only), chunk-contiguous
    i16_per_col = 8
    idxp = np.empty((P.cores, 128, P.tcols * i16_per_col), dtype=np.int16)
    for c in range(P.cores):
        off = 0
        cb = 0
        for ci, (_w0, cl) in enumerate(chunks):
            nc_ = sum(cl)
            n16 = nc_ * i16_per_col
            idxp[c][:, off:off + n16] = _wrap16(idxg[c][:, cb:cb + nc_])
            off += n16
            cb += nc_

    xf = np.asarray(x, dtype=np.float32)
    xts = []
    for c in range(P.cores):
        lo = c * P.nloc
        hi = min((c + 1) * P.nloc, N_NODES)
        xt = np.zeros((IN_CH, P.nloc), dtype=np.float32)
        xt[:, :hi - lo] = xf[lo:hi].T
        xts.append(xt)

    Wt = np.ascontiguousarray(np.asarray(W, dtype=np.float32).T)  # [128,64]
    a_np = np.asarray(a, dtype=np.float32)
    a_src = a_np[0, :, :HEAD_DIM]
    a_dst = a_np[0, :, HEAD_DIM:]
    A_src = (Wt.reshape(IN_CH, HEADS, HEAD_DIM) * a_src[None]).sum(-1)
    A_dst = (Wt.reshape(IN_CH, HEADS, HEAD_DIM) * a_dst[None]).sum(-1)
    rhs_ext = np.ascontiguousarray(
        np.concatenate([Wt, A_src, A_dst], axis=1), dtype=np.float32)
    iota = np.ascontiguousarray(np.broadcast_to(
        np.arange(128, dtype=np.float16), (128, 128)))
    iotap = np.ascontiguousarray(
        np.arange(128, dtype=np.float16).reshape(128, 1))

    in_maps = []
    for c in range(P.cores):
        in_maps.append(dict(
            xT=xts[c], rhs_ext=rhs_ext, iota=iota, iotap=iotap,
            idxp=idxp[c], dstc=dstc[c], dstcT=dstcT[c], wc=wc[c]))
    return in_maps


def _build_program(P, num_devices=None, qmap=None):
    REPS = int(os.environ.get("K3_REPS", "1"))
    ABL = set(os.environ.get("K3_ABL", "").split(","))
    REPCC = int(os.environ.get("K3_REPCC", "1"))
    ACCD = F16 if os.environ.get("K3_F16ACC", "1") == "1" else F32
    ND = num_devices or P.cores
    MW, PG, KC = P.MW, P.PG, P.KC
    nch = len(P.chunks)
    scratch = int(os.environ.get("K3_SCRATCH", "49152"))

    nc = bacc.Bacc("TRN2", target_bir_lowering=False, debug=False,
                   enable_asserts=False, num_devices=ND,
                   dynamic_dma_scratch_size=scratch, num_swdge_queues=int(os.environ.get("K3_NSWQ", "4")))
    xT_d = nc.dram_tensor("xT", [IN_CH, P.nloc], F32, kind="ExternalInput")
    re_d = nc.dram_tensor("rhs_ext", [IN_CH, 72], F32, kind="ExternalInput")
    io_d = nc.dram_tensor("iota", [128, 128], F16, kind="ExternalInput")
    idxp_d = nc.dram_tensor("idxp", [128, P.tcols * 16], I16,
                            kind="ExternalInput")
    dstc_d = nc.dram_tensor("dstc", [128, P.tcols], F16,
                            kind="ExternalInput")
    wc_d = nc.dram_tensor("wc", [128, P.tcols], F16, kind="ExternalInput")

    tab_d = nc.dram_tensor("tab", [P.nloc, ELEM], F16, kind="Internal")
    sdT_d = nc.dram_tensor("sdT", [128, P.lwins * 4], F32, kind="Internal")
    sdF_d = nc.dram_tensor("sdF", [128 * ND, P.lwins * 4], F32,
                           kind="Internal", addr_space="Shared")
    sd256_d = nc.dram_tensor("sd256", [P.nchunks_p * MW * 128, 64], F32,
                             kind="Internal")
    acc_d = nc.dram_tensor("acc", [P.acc_rows, 68], ACCD, kind="Internal")
    red_d = nc.dram_tensor("red", [P.red_rows, 68], ACCD, kind="Internal")
    out_d = nc.dram_tensor("out", [P.red_rows, 64], F32,
                           kind="ExternalOutput")

    groups = [list(range(ND))]

    with tile.TileContext(nc) as tc, ExitStack() as ctx:
        const = ctx.enter_context(tc.tile_pool(name="const", bufs=1))
        iota_t = const.tile([128, 128], F16)
        nc.sync.dma_start(out=iota_t[:], in_=io_d[:])
        re_t = const.tile([128, 72], F32)
        nc.sync.dma_start(out=re_t[:], in_=re_d[:])
        dstc_t = const.tile([128, P.tcols], F16)
        nc.sync.dma_start(out=dstc_t[:], in_=dstc_d[:])
        wc_t = const.tile([128, P.tcols], F16)
        nc.sync.dma_start(out=wc_t[:], in_=wc_d[:])
        sdall = const.tile([128, ND, P.lwins * 4], F32)

        # ---- phase 1: tab rows (permuted) = [h | s_src | junk] fp16 ----
        with tc.tile_pool(name="xload", bufs=1) as xp, \
             tc.tile_pool(name="hstage", bufs=3) as hp, \
             tc.tile_pool(name="psh", bufs=2, space="PSUM") as php:
            xt = xp.tile([128, P.nloc], F32, tag="xt")
            nc.sync.dma_start(out=xt[:], in_=xT_d[:])
            sdstage = xp.tile([128, P.lwins, 4], F32, tag="sds")
            for g7 in range(P.lwins // P.p1g):
                ph = php.tile([128, P.p1g, 72], F32, tag="ph")
                for j in range(P.p1g):
                    jw = g7 * P.p1g + j
                    nc.tensor.matmul(
                        out=ph[:, j, :], lhsT=xt[:, jw * 128:(jw + 1) * 128],
                        rhs=re_t[:], start=True, stop=True)
                hs = hp.tile([128, P.p1g, ELEM], F16, tag="hs")
                nc.vector.memset(hs[:, :, 68:ELEM], 0.0)
                nc.vector.tensor_copy(out=hs[:, :, 0:68],
                                      in_=ph[:, :, 0:68])
                nc.vector.tensor_copy(
                    out=sdstage[:, g7 * P.p1g:(g7 + 1) * P.p1g, :],
                    in_=ph[:, :, 68:72])
                dst_ap = _make_ap(
                    tab_d[:], g7 * P.p1g * 128 * ELEM,
                    [[P.p1g * ELEM, 128], [1, P.p1g * ELEM]])
                nc.sync.dma_start(
                    out=dst_ap, in_=hs[:].rearrange("p j e -> p (j e)"))
            nc.sync.dma_start(
                out=sdT_d[:],
                in_=sdstage[:].rearrange("p j e -> p (j e)"))

        nc.gpsimd.collective_compute(
            "AllGather", mybir.AluOpType.bypass, groups,
            ins=[sdT_d[:]], outs=[sdF_d[:]])
        src_ap = _make_ap(sdF_d[:], 0,
                          [[P.lwins * 4, 128], [128 * P.lwins * 4, ND],
                           [1, P.lwins * 4]])
        nc.sync.dma_start(out=sdall[:], in_=src_ap)

        # sd256 chunk blocks: row p*MW + j = s_dst(node w0+j @ part p)
        with tc.tile_pool(name="sdb", bufs=3) as sdb:
            for ci, (w0, cl) in enumerate(P.chunks):
                nw = len(cl)
                st = sdb.tile([128, MW, 64], F32, tag="st")
                nc.vector.memset(st[:], 0.0)
                j = 0
                while j < nw:
                    wg = w0 + j
                    k8, jw = divmod(wg, P.lwins)
                    span = min(nw - j, P.lwins - jw)
                    sap = _make_ap(
                        sdall[:], k8 * P.lwins * 4 + jw * 4,
                        [list(sdall[:].ap[0]), [4, span], [1, 4]])
                    nc.vector.tensor_copy(out=st[:, j:j + span, 0:4],
                                          in_=sap)
                    j += span
                dst_ap = _make_ap(
                    sd256_d[:], ci * MW * 128 * 64,
                    [[MW * 64, 128], [1, MW * 64]])
                nc.sync.dma_start(
                    out=dst_ap, in_=st[:].rearrange("p w e -> p (w e)"))

        # ---------------- phase 2: edges ----------------
        p2ctx = ExitStack()
        sbi = p2ctx.enter_context(tc.tile_pool(name="edgei", bufs=3))
        sbg = p2ctx.enter_context(tc.tile_pool(name="edgeg", bufs=3))
        sbs = p2ctx.enter_context(tc.tile_pool(name="edges", bufs=3))
        wb = p2ctx.enter_context(tc.tile_pool(name="winb", bufs=3))
        psa = p2ctx.enter_context(tc.tile_pool(name="psa", bufs=3,
                                               space="PSUM"))
        pse = p2ctx.enter_context(tc.tile_pool(name="pse", bufs=3,
                                               space="PSUM"))

        QROT = int(os.environ.get("K3_QROT", "0"))
        NQ = int(os.environ.get("K3_NSWQ", "4"))
        swq = [0]
        for _rp in range(REPS if "nophase2" not in ABL else 0):
            colptr = 0
            idxoff = 0
            for ci, (w0, cl) in enumerate(P.chunks):
                nw = len(cl)
                cols_c = sum(cl)
                nidx = cols_c * 128
                n16 = cols_c * 8

                idx_t = sbi.tile([128, KC * 8], I16, tag="idx")
                nc.sync.dma_start(
                    out=idx_t[:, 0:n16],
                    in_=idxp_d[:, idxoff:idxoff + n16])
                dct = sbi.tile([128, KC, 128], mybir.dt.int8, tag="dct")
                nc.sync.dma_start(
                    out=dct[:, 0:cols_c, :].rearrange("p c d -> p (c d)"),
                    in_=dstcT_d[:, colptr * 128:(colptr + cols_c) * 128])
                GOPC = int(os.environ.get("K3_GOPC", "22")) or cols_c
                g = sbg.tile([128, KC, ELEM], F16, tag="g")
                if "nogather" in ABL:
                    nc.vector.memset(g[:, 0:1, 0:4], 0.0)
                for o0 in range(0, cols_c if "nogather" not in ABL else 0,
                                GOPC):
                    on = min(GOPC, cols_c - o0)
                    nc.gpsimd.dma_gather(
                        g[:, o0:o0 + on, :], tab_d[:],
                        idx_t[:, o0 * 8:(o0 + on) * 8],
                        on * 128, on * 128, ELEM,
                        queue_num=(qmap[swq[0] % len(qmap)] if qmap
                                   else (swq[0] * QROT) % NQ),
                        single_packet=False)
                    swq[0] += 1
                # ohT[d, c, p] = (dstc[p, c] == d); expansion matmuls give
                # per-edge s_dst in PSUM: sde_ps[p, c, :] = sdall[dstc[p,c]]
                ohT = sbs.tile([128, KC, 128], F16, tag="ohT")
                sde_ps = pse.tile([128, KC, 4], F32, tag="sdeps")
                if "noexp" in ABL:
                    nc.vector.memset(ohT[:, 0:1, 0:4], 0.0)
                    nc.vector.memset(sde_ps[:, 0:1, :], 0.0)
                else:
                    nc.vector.tensor_tensor(
                        out=ohT[:, 0:cols_c, :],
                        in0=iop_t[:].to_broadcast([128, cols_c, 128]),
                        in1=dct[:, 0:cols_c, :],
                        op=mybir.AluOpType.is_equal)
                    cb2 = 0
                    for j2, ncw2 in enumerate(cl):
                        wg = w0 + j2
                        k8, jw = divmod(wg, P.lwins)
                        for j3 in range(ncw2):
                            nc.tensor.matmul(
                                out=sde_ps[:, cb2 + j3, :],
                                lhsT=ohT[:, cb2 + j3, :],
                                rhs=sdallh[:, k8, jw * 4:(jw + 1) * 4],
                                start=True, stop=True)
                        cb2 += ncw2

                oh = wb.tile([128, KC, 128], F16, tag="oh")
                if "nooh" in ABL:
                    nc.vector.memset(oh[:, 0:1, 0:4], 0.0)
                else:
                    nc.vector.tensor_tensor(
                        out=oh[:, 0:cols_c, :],
                        in0=_bcast_dim(iota_t[:], 1, cols_c),
                        in1=dstc_t[:, colptr:colptr + cols_c].to_broadcast(
                            [128, cols_c, 128]),
                        op=mybir.AluOpType.is_equal)

                logit = wb.tile([128, KC, 4], F16, tag="logit")
                nc.vector.tensor_add(
                    out=logit[:, 0:cols_c, :],
                    in0=g[:, 0:cols_c, 64:68],
                    in1=sde_ps[:, 0:cols_c, :])
                nc.vector.scalar_tensor_tensor(
                    out=logit[:, 0:cols_c, :], in0=logit[:, 0:cols_c, :],
                    scalar=NEG, in1=logit[:, 0:cols_c, :],
                    op0=mybir.AluOpType.mult, op1=mybir.AluOpType.max)
                nc.vector.tensor_mul(
                    out=logit[:, 0:cols_c, :], in0=logit[:, 0:cols_c, :],
                    in1=wc_t[:, colptr:colptr + cols_c].to_broadcast(
                        [128, cols_c, 4]))
                p = wb.tile([128, KC, 4], F16, tag="p")
                if "noact" in ABL:
                    nc.vector.tensor_copy(out=p[:, 0:cols_c, :],
                                          in_=logit[:, 0:cols_c, :])
                else:
                    nc.scalar.activation(p[:, 0:cols_c, :],
                                         logit[:, 0:cols_c, :],
                                         mybir.ActivationFunctionType.Exp)

                pay = wb.tile([128, KC, 68], F16, tag="pay")
                pv = p[:, 0:cols_c, :].to_broadcast([128, cols_c, 4, 16])
                gv = g[:, 0:cols_c, 0:64].rearrange(
                    "p k (h d) -> p k h d", d=16)
                ov = pay[:, 0:cols_c, 0:64].rearrange(
                    "p k (h d) -> p k h d", d=16)
                nc.vector.tensor_mul(out=ov, in0=gv, in1=pv)
                nc.vector.tensor_copy(out=pay[:, 0:cols_c, 64:68],
                                      in_=p[:, 0:cols_c, :])

                stage = wb.tile([128, MW, 68], ACCD, tag="stage")
                if nw < MW:
                    nc.vector.memset(stage[:, nw:MW, :], 0.0)
                for g0 in range(0, nw, PG):
                    gn = min(PG, nw - g0)
                    acc_ps = psa.tile([128, PG, 68], F32, tag="acc")
                    cb = sum(cl[:g0])
                    if "nomm" in ABL:
                        nc.vector.memset(acc_ps[:, 0:1, 0:4], 0.0)
                    for wl in range(gn if "nomm" not in ABL else 0):
                        ncw = cl[g0 + wl]
                        for j in range(ncw):
                            nc.tensor.matmul(
                                out=acc_ps[:, wl, :],
                                lhsT=oh[:, cb + j, :],
                                rhs=pay[:, cb + j, :],
                                start=(j == 0), stop=(j == ncw - 1))
                        cb += ncw
                    nc.vector.tensor_copy(
                        out=stage[:, g0:g0 + gn, :],
                        in_=acc_ps[:, 0:gn, :])
                dst_ap = _make_ap(
                    acc_d[:], ci * MW * 128 * 68,
                    [[MW * 68, 128], [1, MW * 68]])
                nc.scalar.dma_start(
                    out=dst_ap, in_=stage[:].rearrange("p w e -> p (w e)"))
                colptr += cols_c
                idxoff += n16

        p2ctx.close()

        # ---------------- ReduceScatter ----------------
        SEGB = ND * MW * 128
        OUTB = MW * 128
        nseg = P.acc_rows // SEGB
        for _rc in range(REPCC):
            if ND == 1:
                nc.sync.dma_start(out=red_d[:], in_=acc_d[0:P.red_rows, :])
            else:
                for s in range(nseg):
                    nc.gpsimd.collective_compute(
                        "ReduceScatter", mybir.AluOpType.add, groups,
                        ins=[acc_d[s * SEGB:(s + 1) * SEGB, :]],
                        outs=[red_d[s * OUTB:(s + 1) * OUTB, :]])

        # ---------------- final: out = num / (den + eps) ----------------
        with tc.tile_pool(name="fin", bufs=2) as fp:
            for fb in range(P.red_rows // (MW * 128)):
                src2 = _make_ap(red_d[:], fb * MW * 128 * 68,
                                [[MW * 68, 128], [1, MW * 68]])
                rt = fp.tile([128, MW, 68], ACCD, tag="rt")
                nc.sync.dma_start(
                    out=rt[:].rearrange("p w e -> p (w e)"), in_=src2)
                rec = fp.tile([128, MW, 4], F32, tag="rec")
                nc.vector.tensor_scalar_add(out=rec[:], in0=rt[:, :, 64:68],
                                            scalar1=EPS)
                nc.vector.reciprocal(out=rec[:], in_=rec[:])
                ot = fp.tile([128, MW, 64], F32, tag="ot")
                nc.vector.tensor_mul(
                    out=ot[:].rearrange("p k (h d) -> p k h d", d=16),
                    in0=rt[:, :, 0:64].rearrange("p k (h d) -> p k h d",
                                                 d=16),
                    in1=rec[:].to_broadcast([128, MW, 4, 16]))
                dst2 = _make_ap(out_d[:], fb * MW * 128 * 64,
                                [[MW * 64, 128], [1, MW * 64]])
                nc.sync.dma_start(
                    out=dst2, in_=ot[:].rearrange("p w e -> p (w e)"))

    nc.compile()
    return nc


def _gather_lane_qmap(nc):
    """Emission-ordered SWDGE lane assignment -> queue map (lane %% 4)."""
    from concourse.tile_scheduler import PROC_NAME_TO_IDX
    idx_to_proc = {v: k for k, v in PROC_NAME_TO_IDX.items()}
    lanes = []
    for blk in nc.m.functions[0].blocks:
        for ins in blk.instructions:
            if type(ins).__name__ == "InstDMAGatherAnt":
                proc = idx_to_proc.get(ins.bass_scheduled_proc, "")
                assert proc.startswith("DMASW"), proc
                lanes.append((int(ins.name.split("-")[1]),
                              int(proc[5:]) % 4))
    lanes.sort()
    return [q for _, q in lanes]


def build_program_tuned(P, num_devices=None):
    """Two-pass build: discover Tile's DMASW lane order, re-emit with
    queue_num matched to lane %% 4 (consistent sem/queue pairing at
    4-queue parallelism)."""
    nc0 = _build_program(P, num_devices)
    qmap = _gather_lane_qmap(nc0)
    return _build_program(P, num_devices, qmap=qmap)


def _build_base(P, num_devices=None):
    """I/O-identical near-empty program for dispatch-overhead calibration."""
    ND = num_devices or P.cores
    nc = bacc.Bacc("TRN2", target_bir_lowering=False, debug=False,
                   enable_asserts=False, num_devices=ND)
    xT_d = nc.dram_tensor("xT", [IN_CH, P.nloc], F32, kind="ExternalInput")
    nc.dram_tensor("rhs_ext", [IN_CH, 72], F32, kind="ExternalInput")
    nc.dram_tensor("iota", [128, 128], F16, kind="ExternalInput")
    nc.dram_tensor("idxp", [128, P.tcols * 8], I16, kind="ExternalInput")
    nc.dram_tensor("dstc", [128, P.tcols], F16, kind="ExternalInput")
    nc.dram_tensor("dstcT", [128, P.tcols * 128], mybir.dt.int8,
                   kind="ExternalInput")
    nc.dram_tensor("iotap", [128, 1], F16, kind="ExternalInput")
    nc.dram_tensor("wc", [128, P.tcols], F16, kind="ExternalInput")
    out_d = nc.dram_tensor("out", [P.red_rows, 64], F32,
                           kind="ExternalOutput")
    with tile.TileContext(nc) as tc, ExitStack() as ctx:
        sb = ctx.enter_context(tc.tile_pool(name="sb", bufs=1))
        f = sb.tile([128, 64], F32)
        nc.sync.dma_start(out=f[:], in_=xT_d[:, 0:64])
        nb = P.red_rows // 128
        dst = _make_ap(out_d[:], 0, [[64, 128], [0, nb], [1, 64]])
        nc.sync.dma_start(out=dst, in_=_bcast_dim(f[:], 1, nb))
    nc.compile()
    return nc


def _assemble(P, parts):
    """Per-core [red_rows, 64] outputs -> full [N_NODES, 64]."""
    MW = P.MW
    full = np.zeros((P.wins * 128, 64), dtype=parts[0].dtype)
    nb = P.nchunks_p // P.cores
    for k in range(P.cores):
        arr = parts[k].reshape(nb, 128, MW, 64)
        for s in range(nb):
            ci = P.cores * s + k
            if ci >= len(P.chunks):
                continue
            w0, cl = P.chunks[ci]
            nw = len(cl)
            blk = arr[s, :, 0:nw, :].transpose(1, 0, 2)   # [nw, 128, 64]
            full[w0 * 128:(w0 + nw) * 128] = blk.reshape(nw * 128, 64)
    return full[:N_NODES]


def kernel(x, edge_index, edge_weight, W, a):
    global LAST_EXEC_NS, LAST_NC, LAST_IN_MAPS, LAST_PLAN
    P = Plan()
    t0 = time.time()
    in_maps = _host_prep(P, x, edge_index, edge_weight, W, a)
    t1 = time.time()
    nc = build_program_tuned(P)
    LAST_NC = nc
    LAST_IN_MAPS = in_maps
    LAST_PLAN = P
    t2 = time.time()
    res = bass_utils.run_bass_kernel_spmd(
        nc, in_maps, core_ids=list(range(P.cores)))
    t3 = time.time()
    print(f"[kernel3] host_prep {t1-t0:.1f}s  build+compile {t2-t1:.1f}s  "
          f"exec(all-in) {t3-t2:.1f}s  tcols={P.tcols} nch={len(P.chunks)}")
    LAST_EXEC_NS = res.exec_time_ns
    parts = [res.results[c]["out"] for c in range(P.cores)]
    full = _assemble(P, parts)
    return np.ascontiguousarray(full)
